# revision 6
# baseline (speedup 1.0000x reference)
"""GQA causal attention (RoPE) for TRN2, 8-core data+tensor parallel.

Sharding: core c in [0,8) handles batch b = c//4 and kv-head group g = c%4
(kv heads {2g, 2g+1}, q heads {4g..4g+3}).  wq/wk/wv column-sharded,
wo row-sharded by head group; host sums the 4 partial wo outputs per batch.

Device layouts (feature-major, "T" = transposed vs reference):
  xT   [DIM, S]      activations, d on partitions
  QT   [128, S]      per q head (head_dim on partitions)
  KT   [128, S]      per kv head
  V    [128k, 256]   natural (position on partitions), 16 k-tiles
  scoresT[k, q]      so softmax denominator is a partition-dim sum (ones matmul)
  attnT [128d, S]    per head -> wo matmul -> outT [DIM, S] (host transposes)

RoPE on [d, s] tiles: out = qt * C + swap_pairs(qt) * S~, with the pair swap
done by a permutation matmul on the PE and C/S~ tables prebuilt on host.
"""

import json

import numpy as np
import ml_dtypes

import concourse.bass as bass
import concourse.mybir as mybir
import concourse.tile as tile
import concourse.bass2jax as bass2jax
import concourse.bass_utils as bass_utils
from concourse.bass_utils import run_bass_kernel_spmd


def _split_waits(bir_json: bytes) -> bytes:
    """This walrus build accepts at most ONE sync-wait per instruction (any
    opcode). Tile emits up to ~11. Hoist excess waits onto single-wait Drain
    fillers inserted just before the instruction on the same engine —
    same-engine program order makes this semantically identical."""
    j = json.loads(bir_json)
    changed = False
    for fn in j["functions"]:
        for b in fn["blocks"]:
            out = []
            for ins in b["instructions"]:
                si = ins.get("sync_info")
                ow = si.get("on_wait") if si else None
                if ow and len(ow) > 1:
                    changed = True
                    for k, w in enumerate(ow[:-1]):
                        out.append({
                            "debug": ins.get("debug", 0),
                            "engine": ins["engine"],
                            "ins": [], "outs": [],
                            "name": f"{ins['name']}-w{k}",
                            "opcode": "Drain",
                            "is_reset_sema": False,
                            "sync_info": {"on_update": [], "on_wait": [w]},
                        })
                    si["on_wait"] = [ow[-1]]
                out.append(ins)
            b["instructions"] = out
    return json.dumps(j).encode() if changed else bir_json


_ORIG_COMPILE = bass_utils.compile_bir_kernel


def _patched_compile(bir_json, tmpdir, neff_name="file.neff"):
    return _ORIG_COMPILE(_split_waits(bir_json), tmpdir, neff_name=neff_name)


if getattr(bass2jax.compile_bir_kernel, "__name__", "") != "_patched_compile":
    bass2jax.compile_bir_kernel = _patched_compile
    bass_utils.compile_bir_kernel = _patched_compile

BF16 = mybir.dt.bfloat16
F32 = mybir.dt.float32
Exp = mybir.ActivationFunctionType.Exp
Ln = mybir.ActivationFunctionType.Ln

B, S, DIM = 2, 2048, 2048
N_HEADS, N_KV_HEADS = 16, 8
HEAD_DIM, HALF = 128, 64
N_CORES = 8
QH, KVH = 4, 2            # q / kv heads per core
QW, KW = QH * HEAD_DIM, KVH * HEAD_DIM   # 512, 256
SCALE = 1.0 / float(np.sqrt(HEAD_DIM))

DT = DIM // 128           # 16 contraction tiles for projections
NSC = S // 512            # 4 s-chunks
NKT = S // 128            # 16 k tiles
NET = DIM // 128          # 16 output-feature tiles

_BUILT = {}


def _build(nc):
    xt = nc.dram_tensor("xt", [DIM, S], BF16, kind="ExternalInput").ap()
    wq = nc.dram_tensor("wq", [DIM, QW], BF16, kind="ExternalInput").ap()
    wk = nc.dram_tensor("wk", [DIM, KW], BF16, kind="ExternalInput").ap()
    wv = nc.dram_tensor("wv", [DIM, KW], BF16, kind="ExternalInput").ap()
    wo = nc.dram_tensor("wo", [QW, DIM], BF16, kind="ExternalInput").ap()
    cosb = nc.dram_tensor("cosb", [HEAD_DIM, S], BF16, kind="ExternalInput").ap()
    sinb = nc.dram_tensor("sinb", [HEAD_DIM, S], F32, kind="ExternalInput").ap()
    pswp = nc.dram_tensor("pswp", [HEAD_DIM, HEAD_DIM], BF16, kind="ExternalInput").ap()
    tri = nc.dram_tensor("tri", [HEAD_DIM, HEAD_DIM], BF16, kind="ExternalInput").ap()
    ones = nc.dram_tensor("ones", [HEAD_DIM, 1], BF16, kind="ExternalInput").ap()
    outT = nc.dram_tensor("outT", [DIM, S], F32, kind="ExternalOutput").ap()
    # DRAM bounce buffers for partition-broadcast of per-position reciprocals
    rscr = [nc.dram_tensor(f"rscr{i}", [1, 512], F32).ap() for i in range(NSC * QH)]

    with tile.TileContext(nc) as tc:
        with (
            tc.tile_pool(name="persist", bufs=1) as pp,
            tc.tile_pool(name="trans", bufs=2) as tp,
        ):
            # ---- constants / weights resident in SBUF ----
            cos_sb = pp.tile([HEAD_DIM, S], BF16, tag="cos", name="cos_sb")
            nc.sync.dma_start(cos_sb[:], cosb[:])
            sin_sb = pp.tile([HEAD_DIM, S], F32, tag="sin", name="sin_sb")
            nc.sync.dma_start(sin_sb[:], sinb[:])
            pswp_sb = pp.tile([HEAD_DIM, HEAD_DIM], BF16, tag="pswp", name="pswp_sb")
            nc.sync.dma_start(pswp_sb[:], pswp[:])
            tri_sb = pp.tile([HEAD_DIM, HEAD_DIM], BF16, tag="tri", name="tri_sb")
            nc.sync.dma_start(tri_sb[:], tri[:])
            ones_sb = pp.tile([HEAD_DIM, 1], BF16, tag="ones", name="ones_sb")
            nc.sync.dma_start(ones_sb[:], ones[:])

            wq_sb, wk_sb, wv_sb = [], [], []
            for d in range(DT):
                t = pp.tile([128, QW], BF16, tag=f"wq{d}", name=f"wq_sb{d}")
                nc.sync.dma_start(t[:], wq[d * 128:(d + 1) * 128, :])
                wq_sb.append(t)
                t = pp.tile([128, KW], BF16, tag=f"wk{d}", name=f"wk_sb{d}")
                nc.sync.dma_start(t[:], wk[d * 128:(d + 1) * 128, :])
                wk_sb.append(t)
                t = pp.tile([128, KW], BF16, tag=f"wv{d}", name=f"wv_sb{d}")
                nc.sync.dma_start(t[:], wv[d * 128:(d + 1) * 128, :])
                wv_sb.append(t)
            wo_sb = []
            for h in range(QH):
                t = pp.tile([128, DIM], BF16, tag=f"wo{h}", name=f"wo_sb{h}")
                nc.sync.dma_start(t[:], wo[h * 128:(h + 1) * 128, :])
                wo_sb.append(t)

            # persistent intermediates
            qtu = [pp.tile([128, S], BF16, tag=f"qtu{h}", name=f"qtu{h}") for h in range(QH)]
            ktu = [pp.tile([128, S], BF16, tag=f"ktu{k}", name=f"ktu{k}") for k in range(KVH)]
            qtr = [pp.tile([128, S], BF16, tag=f"qtr{h}", name=f"qtr{h}") for h in range(QH)]
            ktr = [pp.tile([128, S], BF16, tag=f"ktr{k}", name=f"ktr{k}") for k in range(KVH)]
            v_sb = [pp.tile([128, KW], BF16, tag=f"v{st}", name=f"v{st}") for st in range(NKT)]
            attnT = [pp.tile([128, S], BF16, tag=f"attnT{h}", name=f"attnT{h}") for h in range(QH)]

            # ================= Phase A: projections =================
            with (
                tc.tile_pool(name="pA", bufs=3, space="PSUM") as pA,
                tc.tile_pool(name="pAv", bufs=2, space="PSUM") as pAv,
                tc.tile_pool(name="prm", bufs=2, space="PSUM") as prm,
            ):
                for sc in range(NSC):
                    ssl = slice(sc * 512, (sc + 1) * 512)
                    xts = []
                    for d in range(DT):
                        xt_t = tp.tile([128, 512], BF16, tag="xts", bufs=20, name=f"xts{sc}_{d}")
                        nc.sync.dma_start(xt_t[:], xt[d * 128:(d + 1) * 128, ssl])
                        xts.append(xt_t)
                    for h in range(QH):
                        ps = pA.tile([128, 512], F32, tag="qk", name=f"qps{sc}_{h}")
                        for d in range(DT):
                            nc.tensor.matmul(ps[:], wq_sb[d][:, h * 128:(h + 1) * 128],
                                             xts[d][:], start=(d == 0), stop=(d == DT - 1))
                        nc.vector.tensor_copy(qtu[h][:, ssl], ps[:])
                    for kv in range(KVH):
                        ps = pA.tile([128, 512], F32, tag="qk", name=f"kps{sc}_{kv}")
                        for d in range(DT):
                            nc.tensor.matmul(ps[:], wk_sb[d][:, kv * 128:(kv + 1) * 128],
                                             xts[d][:], start=(d == 0), stop=(d == DT - 1))
                        nc.vector.tensor_copy(ktu[kv][:, ssl], ps[:])
                    for sv in range(4):
                        st = sc * 4 + sv
                        ps = pAv.tile([128, KW], F32, tag="v", name=f"vps{st}")
                        for d in range(DT):
                            nc.tensor.matmul(ps[:], xts[d][:, sv * 128:(sv + 1) * 128],
                                             wv_sb[d][:], start=(d == 0), stop=(d == DT - 1))
                        nc.vector.tensor_copy(v_sb[st][:], ps[:])

                # ================= RoPE =================
                for src, dst in list(zip(qtu, qtr)) + list(zip(ktu, ktr)):
                    for c4 in range(NSC):
                        sl = slice(c4 * 512, (c4 + 1) * 512)
                        shp = prm.tile([128, 512], F32, tag="shp", name="shp")
                        nc.tensor.matmul(shp[:], pswp_sb[:], src[:, sl], start=True, stop=True)
                        t1 = tp.tile([128, 512], BF16, tag="t1", bufs=3, name="rope_t1")
                        nc.vector.tensor_mul(t1[:], src[:, sl], cos_sb[:, sl])
                        t2 = tp.tile([128, 512], BF16, tag="t2", bufs=3, name="rope_t2")
                        nc.vector.tensor_mul(t2[:], shp[:], sin_sb[:, sl])
                        nc.vector.tensor_add(dst[:, sl], t1[:], t2[:])

            # ================= Phase B/C: attention + out proj =================
            with (
                tc.tile_pool(name="scp", bufs=2, space="PSUM") as scp,
                tc.tile_pool(name="attnp", bufs=2, space="PSUM") as attnp,
                tc.tile_pool(name="denp", bufs=2, space="PSUM") as denp,
                tc.tile_pool(name="wop", bufs=2, space="PSUM") as wop,
            ):
                for qc in range(NSC):
                    qsl = slice(qc * 512, (qc + 1) * 512)
                    for h in range(QH):
                        kv = h // 2
                        attn_ps = attnp.tile([128, 512], F32, tag="attn", name=f"attn{qc}_{h}")
                        den_ps = denp.tile([1, 512], F32, tag="den", name=f"den{qc}_{h}")
                        nkt = 4 * qc + 4
                        for kt in range(nkt):
                            off = max(0, 128 * kt - 512 * qc)
                            span = 512 - off
                            scps = scp.tile([128, 512], F32, tag="sc", name=f"sc{qc}_{h}_{kt}")
                            nc.tensor.matmul(scps[:, :span], ktr[kv][:, kt * 128:(kt + 1) * 128],
                                             qtr[h][:, qc * 512 + off:(qc + 1) * 512],
                                             start=True, stop=True)
                            pt = tp.tile([128, 512], BF16, tag="pt", bufs=4, name=f"pt{qc}_{h}_{kt}")
                            nc.scalar.activation(pt[:, :span], scps[:, :span], Exp, scale=SCALE)
                            if kt >= 4 * qc:  # diagonal block: first 128 cols of span
                                nc.vector.tensor_mul(pt[:, :128], pt[:, :128], tri_sb[:])
                            nc.tensor.matmul(attn_ps[:, off:], v_sb[kt][:, kv * 128:(kv + 1) * 128],
                                             pt[:, :span], start=(kt == 0), stop=(kt == nkt - 1))
                            nc.tensor.matmul(den_ps[:, off:], ones_sb[:], pt[:, :span],
                                             start=(kt == 0), stop=(kt == nkt - 1))
                        lnd = tp.tile([1, 512], F32, tag="lnd", bufs=2, name=f"lnd{qc}_{h}")
                        nc.scalar.activation(lnd[:], den_ps[:], Ln)
                        recip = tp.tile([1, 512], F32, tag="recip", bufs=2, name=f"recip{qc}_{h}")
                        nc.scalar.activation(recip[:], lnd[:], Exp, scale=-1.0)
                        scr = rscr[qc * QH + h]
                        nc.sync.dma_start(scr[:], recip[:])
                        rb = tp.tile([128, 512], F32, tag="rb", bufs=2, name=f"rb{qc}_{h}")
                        bc = bass.AP(tensor=scr.tensor, offset=scr.offset,
                                     ap=[[0, 128]] + list(scr.ap[1:]))
                        nc.sync.dma_start(rb[:], bc)
                        nc.vector.tensor_mul(attnT[h][:, qsl], attn_ps[:], rb[:])
                    for et in range(NET):
                        wo_ps = wop.tile([128, 512], F32, tag="wo", name=f"wops{qc}_{et}")
                        for h in range(QH):
                            nc.tensor.matmul(wo_ps[:], wo_sb[h][:, et * 128:(et + 1) * 128],
                                             attnT[h][:, qsl], start=(h == 0), stop=(h == QH - 1))
                        stage = tp.tile([128, 512], F32, tag="stage", bufs=4, name=f"stage{qc}_{et}")
                        nc.scalar.copy(stage[:], wo_ps[:])
                        nc.sync.dma_start(outT[et * 128:(et + 1) * 128, qsl], stage[:])
    return nc


def get_nc():
    if "nc" not in _BUILT:
        nc = bass.Bass("TRN2", debug=False, enable_asserts=False,
                       num_devices=N_CORES)
        _BUILT["nc"] = _build(nc)
    return _BUILT["nc"]


def prepare_in_maps(x, pos_cos, pos_sin, wq, wk, wv, wo):
    bf = ml_dtypes.bfloat16
    x = np.asarray(x, np.float32)
    pos_cos = np.asarray(pos_cos, np.float32)
    pos_sin = np.asarray(pos_sin, np.float32)
    wq = np.asarray(wq, np.float32)
    wk = np.asarray(wk, np.float32)
    wv = np.asarray(wv, np.float32)
    wo = np.asarray(wo, np.float32)

    pair = np.repeat(np.arange(HALF), 2)          # d -> d//2
    C = pos_cos.T[pair]                           # [128, S]
    Sm = pos_sin.T[pair].copy()                   # [128, S]
    Sm[0::2] *= -1.0                              # even d: -sin, odd d: +sin
    pswap = np.zeros((128, 128), np.float32)
    pswap[np.arange(128), np.arange(128) ^ 1] = 1.0
    tri = np.triu(np.ones((128, 128), np.float32))  # keep j >= i (q >= k)
    ones = np.ones((128, 1), np.float32)

    common = {
        "cosb": C.astype(bf), "sinb": Sm.astype(np.float32),
        "pswp": pswap.astype(bf), "tri": tri.astype(bf), "ones": ones.astype(bf),
    }
    in_maps = []
    for c in range(N_CORES):
        b, g = divmod(c, 4)
        in_maps.append(dict(
            xt=np.ascontiguousarray(x[b].T).astype(bf),
            wq=wq[:, QW * g:QW * (g + 1)].astype(bf),
            wk=wk[:, KW * g:KW * (g + 1)].astype(bf),
            wv=wv[:, KW * g:KW * (g + 1)].astype(bf),
            wo=wo[QW * g:QW * (g + 1), :].astype(bf),
            **common,
        ))
    return in_maps


def gather(results):
    out = np.zeros((B, S, DIM), np.float32)
    for c in range(N_CORES):
        b = c // 4
        out[b] += results[c]["outT"].T
    return out


def run(inputs, trace=False, tmpdir=None):
    nc = get_nc()
    in_maps = prepare_in_maps(**inputs)
    res = run_bass_kernel_spmd(nc, in_maps, list(range(N_CORES)),
                               trace=trace, tmpdir=tmpdir)
    return gather(res.results), res


def kernel(x, pos_cos, pos_sin, wq, wk, wv, wo):
    out, _ = run(dict(x=x, pos_cos=pos_cos, pos_sin=pos_sin,
                      wq=wq, wk=wk, wv=wv, wo=wo))
    return out


# revision 8
# speedup vs baseline: 1.1368x; 1.1368x over previous
"""GQA causal attention (RoPE) for TRN2, 8-core data+tensor parallel.

Sharding: core c in [0,8) handles batch b = c//4 and kv-head group g = c%4
(kv heads {2g, 2g+1}, q heads {4g..4g+3}).  wq/wk/wv column-sharded,
wo row-sharded by head group; host sums the 4 partial wo outputs per batch.

Device layouts (feature-major, "T" = transposed vs reference):
  xT   [DIM, S]      activations, d on partitions
  QT   [128, S]      per q head (head_dim on partitions)
  KT   [128, S]      per kv head
  V    [128k, 256]   natural (position on partitions), 16 k-tiles
  scoresT[k, q]      so softmax denominator is a partition-dim sum (ones matmul)
  attnT [128d, S]    per head -> wo matmul -> outT [DIM, S] (host transposes)

RoPE on [d, s] tiles: out = qt * C + swap_pairs(qt) * S~, with the pair swap
done by a permutation matmul on the PE and C/S~ tables prebuilt on host.
"""

import json

import numpy as np
import ml_dtypes

import concourse.bass as bass
import concourse.mybir as mybir
import concourse.tile as tile
import concourse.bass2jax as bass2jax
import concourse.bass_utils as bass_utils
from concourse.bass_utils import run_bass_kernel_spmd


def _split_waits(bir_json: bytes) -> bytes:
    """This walrus build accepts at most ONE sync-wait per instruction (any
    opcode). Tile emits up to ~11. Hoist excess waits onto single-wait Drain
    fillers inserted just before the instruction on the same engine —
    same-engine program order makes this semantically identical."""
    j = json.loads(bir_json)
    changed = False
    for fn in j["functions"]:
        for b in fn["blocks"]:
            out = []
            for ins in b["instructions"]:
                si = ins.get("sync_info")
                ow = si.get("on_wait") if si else None
                if ow and len(ow) > 1:
                    changed = True
                    for k, w in enumerate(ow[:-1]):
                        out.append({
                            "debug": ins.get("debug", 0),
                            "engine": ins["engine"],
                            "ins": [], "outs": [],
                            "name": f"{ins['name']}-w{k}",
                            "opcode": "Drain",
                            "is_reset_sema": False,
                            "sync_info": {"on_update": [], "on_wait": [w]},
                        })
                    si["on_wait"] = [ow[-1]]
                out.append(ins)
            b["instructions"] = out
    return json.dumps(j).encode() if changed else bir_json


_ORIG_COMPILE = bass_utils.compile_bir_kernel


def _patched_compile(bir_json, tmpdir, neff_name="file.neff"):
    return _ORIG_COMPILE(_split_waits(bir_json), tmpdir, neff_name=neff_name)


if getattr(bass2jax.compile_bir_kernel, "__name__", "") != "_patched_compile":
    bass2jax.compile_bir_kernel = _patched_compile
    bass_utils.compile_bir_kernel = _patched_compile

BF16 = mybir.dt.bfloat16
F32 = mybir.dt.float32
Exp = mybir.ActivationFunctionType.Exp
Ln = mybir.ActivationFunctionType.Ln

B, S, DIM = 2, 2048, 2048
N_HEADS, N_KV_HEADS = 16, 8
HEAD_DIM, HALF = 128, 64
N_CORES = 8
QH, KVH = 4, 2            # q / kv heads per core
QW, KW = QH * HEAD_DIM, KVH * HEAD_DIM   # 512, 256
SCALE = 1.0 / float(np.sqrt(HEAD_DIM))

DT = DIM // 128           # 16 contraction tiles for projections
NSC = S // 512            # 4 s-chunks
NKT = S // 128            # 16 k tiles
NET = DIM // 128          # 16 output-feature tiles

_BUILT = {}


def _build(nc):
    xt = nc.dram_tensor("xt", [DIM, S], BF16, kind="ExternalInput").ap()
    wq = nc.dram_tensor("wq", [DIM, QW], BF16, kind="ExternalInput").ap()
    wk = nc.dram_tensor("wk", [DIM, KW], BF16, kind="ExternalInput").ap()
    wv = nc.dram_tensor("wv", [DIM, KW], BF16, kind="ExternalInput").ap()
    wo = nc.dram_tensor("wo", [QW, DIM], BF16, kind="ExternalInput").ap()
    cosb = nc.dram_tensor("cosb", [HEAD_DIM, S], BF16, kind="ExternalInput").ap()
    sinb = nc.dram_tensor("sinb", [HEAD_DIM, S], F32, kind="ExternalInput").ap()
    pswp = nc.dram_tensor("pswp", [HEAD_DIM, HEAD_DIM], BF16, kind="ExternalInput").ap()
    tri = nc.dram_tensor("tri", [HEAD_DIM, HEAD_DIM], BF16, kind="ExternalInput").ap()
    ones = nc.dram_tensor("ones", [HEAD_DIM, 1], BF16, kind="ExternalInput").ap()
    outT = nc.dram_tensor("outT", [DIM, S], F32, kind="ExternalOutput").ap()
    # DRAM bounce buffers for partition-broadcast of per-position reciprocals
    rscr = [nc.dram_tensor(f"rscr{i}", [1, 512], F32).ap() for i in range(NSC * QH)]

    with tile.TileContext(nc) as tc:
        with (
            tc.tile_pool(name="persist", bufs=1) as pp,
            tc.tile_pool(name="trans", bufs=2) as tp,
        ):
            # ---- constants / weights resident in SBUF ----
            cos_sb = pp.tile([HEAD_DIM, S], BF16, tag="cos", name="cos_sb")
            nc.sync.dma_start(cos_sb[:], cosb[:])
            sin_sb = pp.tile([HEAD_DIM, S], F32, tag="sin", name="sin_sb")
            nc.sync.dma_start(sin_sb[:], sinb[:])
            pswp_sb = pp.tile([HEAD_DIM, HEAD_DIM], BF16, tag="pswp", name="pswp_sb")
            nc.sync.dma_start(pswp_sb[:], pswp[:])
            tri_sb = pp.tile([HEAD_DIM, HEAD_DIM], BF16, tag="tri", name="tri_sb")
            nc.sync.dma_start(tri_sb[:], tri[:])
            ones_sb = pp.tile([HEAD_DIM, 1], BF16, tag="ones", name="ones_sb")
            nc.sync.dma_start(ones_sb[:], ones[:])

            wq_sb, wk_sb, wv_sb = [], [], []
            for d in range(DT):
                t = pp.tile([128, QW], BF16, tag=f"wq{d}", name=f"wq_sb{d}")
                nc.sync.dma_start(t[:], wq[d * 128:(d + 1) * 128, :])
                wq_sb.append(t)
                t = pp.tile([128, KW], BF16, tag=f"wk{d}", name=f"wk_sb{d}")
                nc.sync.dma_start(t[:], wk[d * 128:(d + 1) * 128, :])
                wk_sb.append(t)
                t = pp.tile([128, KW], BF16, tag=f"wv{d}", name=f"wv_sb{d}")
                nc.sync.dma_start(t[:], wv[d * 128:(d + 1) * 128, :])
                wv_sb.append(t)
            wo_sb = []
            for h in range(QH):
                t = pp.tile([128, DIM], BF16, tag=f"wo{h}", name=f"wo_sb{h}")
                nc.sync.dma_start(t[:], wo[h * 128:(h + 1) * 128, :])
                wo_sb.append(t)

            # persistent intermediates
            qtu = [pp.tile([128, S], BF16, tag=f"qtu{h}", name=f"qtu{h}") for h in range(QH)]
            ktu = [pp.tile([128, S], BF16, tag=f"ktu{k}", name=f"ktu{k}") for k in range(KVH)]
            qtr = [pp.tile([128, S], BF16, tag=f"qtr{h}", name=f"qtr{h}") for h in range(QH)]
            ktr = [pp.tile([128, S], BF16, tag=f"ktr{k}", name=f"ktr{k}") for k in range(KVH)]
            v_sb = [pp.tile([128, KW], BF16, tag=f"v{st}", name=f"v{st}") for st in range(NKT)]
            attnT = [pp.tile([128, S], BF16, tag=f"attnT{h}", name=f"attnT{h}") for h in range(QH)]

            # ================= Phase A: projections =================
            with (
                tc.tile_pool(name="pA", bufs=3, space="PSUM") as pA,
                tc.tile_pool(name="pAv", bufs=2, space="PSUM") as pAv,
                tc.tile_pool(name="prm", bufs=2, space="PSUM") as prm,
            ):
                def rope_chunk(src, dst, ssl):
                    shp = prm.tile([128, 512], F32, tag="shp", name="shp")
                    nc.tensor.matmul(shp[:], pswp_sb[:], src[:, ssl], start=True, stop=True)
                    t1 = tp.tile([128, 512], BF16, tag="t1", bufs=3, name="rope_t1")
                    nc.vector.tensor_mul(t1[:], src[:, ssl], cos_sb[:, ssl])
                    t2 = tp.tile([128, 512], BF16, tag="t2", bufs=3, name="rope_t2")
                    nc.vector.tensor_mul(t2[:], shp[:], sin_sb[:, ssl])
                    nc.vector.tensor_add(dst[:, ssl], t1[:], t2[:])

                for sc in range(NSC):
                    ssl = slice(sc * 512, (sc + 1) * 512)
                    xts = []
                    for d in range(DT):
                        xt_t = tp.tile([128, 512], BF16, tag="xts", bufs=20, name=f"xts{sc}_{d}")
                        nc.sync.dma_start(xt_t[:], xt[d * 128:(d + 1) * 128, ssl])
                        xts.append(xt_t)
                    for h in range(QH):
                        ps = pA.tile([128, 512], F32, tag="qk", name=f"qps{sc}_{h}")
                        for d in range(DT):
                            nc.tensor.matmul(ps[:], wq_sb[d][:, h * 128:(h + 1) * 128],
                                             xts[d][:], start=(d == 0), stop=(d == DT - 1))
                        nc.vector.tensor_copy(qtu[h][:, ssl], ps[:])
                        rope_chunk(qtu[h], qtr[h], ssl)
                    for kv in range(KVH):
                        ps = pA.tile([128, 512], F32, tag="qk", name=f"kps{sc}_{kv}")
                        for d in range(DT):
                            nc.tensor.matmul(ps[:], wk_sb[d][:, kv * 128:(kv + 1) * 128],
                                             xts[d][:], start=(d == 0), stop=(d == DT - 1))
                        nc.vector.tensor_copy(ktu[kv][:, ssl], ps[:])
                        rope_chunk(ktu[kv], ktr[kv], ssl)
                    for sv in range(4):
                        st = sc * 4 + sv
                        ps = pAv.tile([128, KW], F32, tag="v", name=f"vps{st}")
                        for d in range(DT):
                            nc.tensor.matmul(ps[:], xts[d][:, sv * 128:(sv + 1) * 128],
                                             wv_sb[d][:], start=(d == 0), stop=(d == DT - 1))
                        nc.vector.tensor_copy(v_sb[st][:], ps[:])

            # ================= Phase B/C: attention + out proj =================
            with (
                tc.tile_pool(name="scp", bufs=3, space="PSUM") as scp,
                tc.tile_pool(name="attnp", bufs=2, space="PSUM") as attnp,
                tc.tile_pool(name="denp", bufs=1, space="PSUM") as denp,
                tc.tile_pool(name="wop", bufs=2, space="PSUM") as wop,
            ):
                def attn_chunk(qc):
                    qsl = slice(qc * 512, (qc + 1) * 512)
                    for h in range(QH):
                        kv = h // 2
                        attn_ps = attnp.tile([128, 512], F32, tag="attn", name=f"attn{qc}_{h}")
                        dac = tp.tile([128, 512], BF16, tag="dac", bufs=2, name=f"dac{qc}_{h}")
                        nkt = 4 * qc + 4
                        for kt in range(nkt):
                            off = max(0, 128 * kt - 512 * qc)
                            span = 512 - off
                            scps = scp.tile([128, 512], F32, tag="sc", name=f"sc{qc}_{h}_{kt}")
                            nc.tensor.matmul(scps[:, :span], ktr[kv][:, kt * 128:(kt + 1) * 128],
                                             qtr[h][:, qc * 512 + off:(qc + 1) * 512],
                                             start=True, stop=True)
                            pt = tp.tile([128, 512], BF16, tag="pt", bufs=6, name=f"pt{qc}_{h}_{kt}")
                            nc.scalar.activation(pt[:, :span], scps[:, :span], Exp, scale=SCALE)
                            if kt >= 4 * qc:  # diagonal block: first 128 cols of span
                                nc.vector.tensor_mul(pt[:, :128], pt[:, :128], tri_sb[:])
                            nc.tensor.matmul(attn_ps[:, off:], v_sb[kt][:, kv * 128:(kv + 1) * 128],
                                             pt[:, :span], start=(kt == 0), stop=(kt == nkt - 1))
                            if kt == 0:
                                nc.vector.tensor_copy(dac[:], pt[:])
                            else:
                                nc.vector.tensor_add(dac[:, off:], dac[:, off:], pt[:, :span])
                        den_ps = denp.tile([1, 512], F32, tag="den", name=f"den{qc}_{h}")
                        nc.tensor.matmul(den_ps[:], ones_sb[:], dac[:], start=True, stop=True)
                        lnd = tp.tile([1, 512], F32, tag="lnd", bufs=2, name=f"lnd{qc}_{h}")
                        nc.scalar.activation(lnd[:], den_ps[:], Ln)
                        recip = tp.tile([1, 512], F32, tag="recip", bufs=2, name=f"recip{qc}_{h}")
                        nc.scalar.activation(recip[:], lnd[:], Exp, scale=-1.0)
                        scr = rscr[qc * QH + h]
                        nc.sync.dma_start(scr[:], recip[:])
                        rb = tp.tile([128, 512], F32, tag="rb", bufs=2, name=f"rb{qc}_{h}")
                        bc = bass.AP(tensor=scr.tensor, offset=scr.offset,
                                     ap=[[0, 128]] + list(scr.ap[1:]))
                        nc.sync.dma_start(rb[:], bc)
                        nc.vector.tensor_mul(attnT[h][:, qsl], attn_ps[:], rb[:])

                def wo_chunk(qc):
                    qsl = slice(qc * 512, (qc + 1) * 512)
                    for et in range(NET):
                        wo_ps = wop.tile([128, 512], F32, tag="wo", name=f"wops{qc}_{et}")
                        for h in range(QH):
                            nc.tensor.matmul(wo_ps[:], wo_sb[h][:, et * 128:(et + 1) * 128],
                                             attnT[h][:, qsl], start=(h == 0), stop=(h == QH - 1))
                        stage = tp.tile([128, 512], F32, tag="stage", bufs=4, name=f"stage{qc}_{et}")
                        nc.scalar.copy(stage[:], wo_ps[:])
                        nc.sync.dma_start(outT[et * 128:(et + 1) * 128, qsl], stage[:])

                # pipeline: emit wo for chunk qc-1 after attention of chunk qc,
                # so the PE stream never stalls on the normalize chain
                for qc in range(NSC):
                    attn_chunk(qc)
                    if qc >= 1:
                        wo_chunk(qc - 1)
                wo_chunk(NSC - 1)
    return nc


def get_nc():
    if "nc" not in _BUILT:
        nc = bass.Bass("TRN2", debug=False, enable_asserts=False,
                       num_devices=N_CORES)
        _BUILT["nc"] = _build(nc)
    return _BUILT["nc"]


def prepare_in_maps(x, pos_cos, pos_sin, wq, wk, wv, wo):
    bf = ml_dtypes.bfloat16
    x = np.asarray(x, np.float32)
    pos_cos = np.asarray(pos_cos, np.float32)
    pos_sin = np.asarray(pos_sin, np.float32)
    wq = np.asarray(wq, np.float32)
    wk = np.asarray(wk, np.float32)
    wv = np.asarray(wv, np.float32)
    wo = np.asarray(wo, np.float32)

    pair = np.repeat(np.arange(HALF), 2)          # d -> d//2
    C = pos_cos.T[pair]                           # [128, S]
    Sm = pos_sin.T[pair].copy()                   # [128, S]
    Sm[0::2] *= -1.0                              # even d: -sin, odd d: +sin
    pswap = np.zeros((128, 128), np.float32)
    pswap[np.arange(128), np.arange(128) ^ 1] = 1.0
    tri = np.triu(np.ones((128, 128), np.float32))  # keep j >= i (q >= k)
    ones = np.ones((128, 1), np.float32)

    common = {
        "cosb": C.astype(bf), "sinb": Sm.astype(np.float32),
        "pswp": pswap.astype(bf), "tri": tri.astype(bf), "ones": ones.astype(bf),
    }
    in_maps = []
    for c in range(N_CORES):
        b, g = divmod(c, 4)
        in_maps.append(dict(
            xt=np.ascontiguousarray(x[b].T).astype(bf),
            wq=wq[:, QW * g:QW * (g + 1)].astype(bf),
            wk=wk[:, KW * g:KW * (g + 1)].astype(bf),
            wv=wv[:, KW * g:KW * (g + 1)].astype(bf),
            wo=wo[QW * g:QW * (g + 1), :].astype(bf),
            **common,
        ))
    return in_maps


def gather(results):
    out = np.zeros((B, S, DIM), np.float32)
    for c in range(N_CORES):
        b = c // 4
        out[b] += results[c]["outT"].T
    return out


def run(inputs, trace=False, tmpdir=None):
    nc = get_nc()
    in_maps = prepare_in_maps(**inputs)
    res = run_bass_kernel_spmd(nc, in_maps, list(range(N_CORES)),
                               trace=trace, tmpdir=tmpdir)
    return gather(res.results), res


def kernel(x, pos_cos, pos_sin, wq, wk, wv, wo):
    out, _ = run(dict(x=x, pos_cos=pos_cos, pos_sin=pos_sin,
                      wq=wq, wk=wk, wv=wv, wo=wo))
    return out
